# revision 1
# baseline (speedup 1.0000x reference)
"""BlockSparseThresLinear Trainium2 kernel.

out = (x masked by 64x64 block-mean(|x|) > 0.8) @ W,  x:[8192,4096] W:[4096,4096] fp32.

Sharding: data-parallel over rows, 8 cores x 1024 rows; W replicated (each
core streams its W copy from HBM exactly once).

Primary path (SPARSE=True): one specialized program per core. The host
computes the exact (f64) block mask for that core's shard; it is baked into
a block-sparse schedule (k-tiles with no active 64x64 cell are skipped) and
also uploaded as a 0/1 element mask so values match the reference exactly.
x and W ship as bf16; x is PE-transposed into a resident SBUF x^T and masked;
phase 2 runs col-packed M=64 bf16 matmuls — per PSUM bank, two paired 64-row
blocks accumulate in disjoint partition halves so their matmuls dual-issue in
different PE column groups (~2x). Block pairs are matched greedily to
maximize co-included k-tiles.

Fallback (on any failure): dense SPMD bf16 kernel with the mask computed on
device in fp32 (exactly equivalent to the reference's mean>0.8 threshold).
"""

import numpy as np

import concourse.bass as bass
import concourse.mybir as mybir
from concourse import tile
from concourse.bass_utils import run_bass_kernel_spmd
from concourse.masks import make_identity
from concourse.vector_clock import ScopedClock

P = 128
BLOCK = 64
N_CORES = 8
# threshold on the *block sum* (4096 elements): exactly fp32(0.8) * 64*64,
# representable exactly in fp32, so sum > THRES_SUM  <=>  fp32(sum/4096) > fp32(0.8)
THRES_SUM = float(np.float32(0.8)) * BLOCK * BLOCK

_f32 = mybir.dt.float32
_f32r = mybir.dt.float32r


def _install_drain_patch():
    """Bundled walrus rejects >1 sync-wait on a Drain; split the TileContext
    final-drain waits across multiple Drain instructions."""

    def _drain_and_barrier(self, tick_clock, wait_clock):
        nc = self.nc
        drain_inst = nc.sync.drain()
        wait_clock.add_sem_waits(
            drain_inst.ins, ScopedClock({None: tick_clock.global_clock})
        )
        si = drain_inst.ins.sync_info
        if si is not None and si.on_wait and len(si.on_wait) > 1:
            waits = list(si.on_wait)
            si.on_wait = waits[:1]
            drain_inst.ins.sync_info = si
            for w in waits[1:]:
                d2 = nc.sync.drain()
                si2 = d2.ins.sync_info
                if si2 is None:
                    si2 = mybir.SyncInfo(on_wait=[w], on_update=[])
                else:
                    si2.on_wait = list(si2.on_wait) + [w]
                d2.ins.sync_info = si2

        nc.all_engine_barrier()
        assert self.sems is not None
        popped = nc._tile_sem_poison_stack.pop()
        assert popped is self._sem_poison
        nc.clear_and_free_semaphores(list(self.sems.allocated().values()))
        nc.all_engine_barrier()

    tile.TileContext._drain_and_barrier = _drain_and_barrier


_install_drain_patch()


def _split_excess_waits(nc: bass.Bass, max_waits: int = 1):
    """Bundled walrus allows only one sync-wait per instruction; move excess
    waits onto same-engine NoOps inserted right before the instruction."""
    ctr = 0
    for fn in nc.m.functions:
        for bb in fn.blocks:
            out = []
            changed = False
            for inst in bb.instructions:
                si = inst.sync_info
                if si is not None and si.on_wait and len(si.on_wait) > max_waits:
                    waits = list(si.on_wait)
                    for w in waits[:-max_waits]:
                        nop = mybir.InstNoOp(name=f"nopw-{ctr}", ins=[], outs=[])
                        ctr += 1
                        nop.engine = inst.engine
                        nop.sync_info = mybir.SyncInfo(on_wait=[w], on_update=[])
                        out.append(nop)
                    si.on_wait = waits[-max_waits:]
                    inst.sync_info = si
                    changed = True
                out.append(inst)
            if changed:
                bb.instructions = out


def build_kernel(rows: int, d_in: int, d_out: int, n_slice: int = 512, repeat: int = 1,
                 mm_dtype: str = "bf16") -> bass.Bass:
    """One-core SPMD program: y[rows, d_out] = mask(x[rows, d_in]) @ w[d_in, d_out]."""
    MT = rows // P           # m-tiles of 128 rows
    KT = d_in // P           # k-tiles of 128
    NT = d_out // n_slice    # n-slices
    KB = d_in // BLOCK       # 64-wide k-blocks per row

    _dt_mm = mybir.dt.bfloat16 if mm_dtype == "bf16" else _f32r
    nc = bass.Bass()
    x = nc.declare_dram_parameter("x", [rows, d_in], _f32, isOutput=False)
    w = nc.declare_dram_parameter("w", [d_in, d_out], _dt_mm, isOutput=False)
    y = nc.declare_dram_parameter("y", [rows, d_out], _f32, isOutput=True)

    with tile.TileContext(nc) as tc:
        with (
            tc.tile_pool(name="consts", bufs=1) as consts,
            tc.tile_pool(name="xin", bufs=2) as xin_pool,
            tc.tile_pool(name="stats", bufs=2) as stats_pool,
            tc.tile_pool(name="xt", bufs=1) as xt_pool,
            tc.tile_pool(name="wld", bufs=6) as w_pool,
            tc.tile_pool(name="outc", bufs=4) as out_pool,
            tc.tile_pool(name="ps", bufs=8, space="PSUM") as ps_pool,
        ):
            # constants
            ident = consts.tile([P, P], _f32)
            make_identity(nc, ident)
            # G[q, p] = 1 if q//64 == p//64 else 0  (block-ones)
            ones_g = consts.tile([P, P], _f32)
            nc.any.memset(ones_g, 0.0)
            nc.any.memset(ones_g[:BLOCK, :BLOCK], 1.0)
            nc.any.memset(ones_g[BLOCK:, BLOCK:], 1.0)

            # resident masked x^T: [128 k-part, MT, KT, 128 m-col] in the
            # matmul dtype (producers must write the matmul dtype directly)
            xt = xt_pool.tile([P, MT, KT, P], _dt_mm)

            # ---- phase 1: mask + transpose ----
            for mt in range(MT):
                x_t = xin_pool.tile([P, d_in], _f32, tag="x_t")
                nc.sync.dma_start(x_t[:], x[mt * P:(mt + 1) * P, :])

                # per-row 64-chunk |x| sums: [128, KB]
                s_t = stats_pool.tile([P, KB], _f32, tag="s_t")
                nc.vector.reduce_sum(
                    s_t[:],
                    x_t.rearrange("p (kb b) -> p kb b", b=BLOCK),
                    axis=mybir.AxisListType.X,
                    apply_absolute_value=True,
                )
                # block sums broadcast back to all 128 partitions:
                # bs[p, kb] = sum_{q: q//64==p//64} s[q, kb]
                bs_ps = ps_pool.tile([P, n_slice], _f32, tag="ps")
                nc.tensor.matmul(
                    bs_ps[:, :KB], ones_g[:], s_t[:], start=True, stop=True
                )
                # mask = bs > THRES_SUM (1.0 / 0.0)
                mask_t = stats_pool.tile([P, KB], _f32, tag="mask_t")
                nc.vector.tensor_scalar(
                    out=mask_t[:],
                    in0=bs_ps[:, :KB],
                    scalar1=THRES_SUM,
                    scalar2=None,
                    op0=mybir.AluOpType.is_gt,
                )
                # x *= mask (broadcast 64-wide); gpsimd — DVE is the phase-1
                # bottleneck and gpsimd is otherwise idle
                nc.gpsimd.tensor_tensor(
                    x_t.rearrange("p (kb b) -> p kb b", b=BLOCK),
                    x_t.rearrange("p (kb b) -> p kb b", b=BLOCK),
                    mask_t[:, :, None].to_broadcast((P, KB, BLOCK)),
                    mybir.AluOpType.mult,
                )
                # transpose masked x tile into resident x^T
                for kt in range(KT):
                    t_ps = ps_pool.tile([P, n_slice], _f32, tag="ps")
                    nc.tensor.transpose(
                        t_ps[:, :P], x_t[:, kt * P:(kt + 1) * P], ident[:]
                    )
                    if kt % 2 == 1:
                        nc.scalar.copy(out=xt[:, mt, kt, :], in_=t_ps[:, :P])
                    else:
                        nc.vector.tensor_copy(out=xt[:, mt, kt, :], in_=t_ps[:, :P])

            # ---- phase 2: matmuls, stream W once ----
            loop = tc.For_i(0, repeat, 1) if repeat > 1 else None
            if loop is not None:
                loop.__enter__()
            for rnt in range(NT):
                nt = rnt % NT
                acc = []
                for mt in range(MT):
                    acc_mt = ps_pool.tile([P, n_slice], _f32, tag="ps", name=f"acc_{rnt}_{mt}")
                    acc.append(acc_mt)
                for kt in range(KT):
                    w_t = w_pool.tile([P, n_slice], _dt_mm, tag="w_t")
                    nc.sync.dma_start(
                        w_t[:],
                        w[kt * P:(kt + 1) * P, nt * n_slice:(nt + 1) * n_slice],
                    )
                    for mt in range(MT):
                        nc.tensor.matmul(
                            acc[mt][:],
                            xt[:, mt, kt, :],
                            w_t[:],
                            start=(kt == 0),
                            stop=(kt == KT - 1),
                        )
                for mt in range(MT):
                    o_t = out_pool.tile([P, n_slice], _f32, tag="o_t")
                    if mt % 4 == 0:
                        nc.vector.tensor_copy(out=o_t[:], in_=acc[mt][:])
                    else:
                        nc.scalar.copy(out=o_t[:], in_=acc[mt][:])
                    nc.sync.dma_start(
                        y[mt * P:(mt + 1) * P, nt * n_slice:(nt + 1) * n_slice],
                        o_t[:],
                    )
            if loop is not None:
                loop.__exit__(None, None, None)
    return nc


def build_sparse(rows: int, d_in: int, d_out: int, cell_mask: np.ndarray,
                 n_slice: int = 512, repeat: int = 1, loop_all: bool = False) -> bass.Bass:
    """Per-core block-sparse bf16 kernel.

    cell_mask: bool [rows//64, d_in//64] — active 64x64 blocks for this core's
    shard. Drives both the baked matmul schedule and the uploaded element mask.
    Inputs: xb [rows, d_in] bf16, w [d_in, d_out] bf16,
    maskt [d_in, rows//64] bf16 (transposed element mask, 0/1).
    """
    _bf16 = mybir.dt.bfloat16
    MT = rows // P
    KT = d_in // P
    NT = d_out // n_slice
    NB = rows // BLOCK          # 64-row blocks (2 per m-tile)

    # schedule: for each m-block, the k-tiles (128 wide = 2 cells) to visit
    klists = []
    for b in range(NB):
        lst = [kt for kt in range(KT) if cell_mask[b, 2 * kt] or cell_mask[b, 2 * kt + 1]]
        if not lst:
            lst = [0]  # x is masked to zero there; keeps psum group well-formed
        klists.append(lst)
    ksets = [set(l) for l in klists]
    # pair blocks to maximize co-inclusion: a pair's PE stream time per
    # n-slice is |union| (co-included k-tiles dual-issue in both col groups)
    unpaired = list(range(NB))
    unpaired.sort(key=lambda b: -len(ksets[b]))
    pairs = []
    while unpaired:
        u = unpaired.pop(0)
        best = max(range(len(unpaired)),
                   key=lambda j: len(ksets[u] & ksets[unpaired[j]]))
        v = unpaired.pop(best)
        pairs.append((u, v))

    nc = bass.Bass()
    xb = nc.declare_dram_parameter("xb", [rows, d_in], _bf16, isOutput=False)
    w = nc.declare_dram_parameter("w", [d_in, d_out], _bf16, isOutput=False)
    maskt = nc.declare_dram_parameter("maskt", [d_in, NB], _bf16, isOutput=False)
    y = nc.declare_dram_parameter("y", [rows, d_out], _f32, isOutput=True)

    with tile.TileContext(nc) as tc:
        with (
            tc.tile_pool(name="consts", bufs=1) as consts,
            tc.tile_pool(name="xin", bufs=2) as xin_pool,
            tc.tile_pool(name="xt", bufs=1) as xt_pool,
            tc.tile_pool(name="wld", bufs=6) as w_pool,
            tc.tile_pool(name="outc", bufs=4) as out_pool,
            tc.tile_pool(name="ps", bufs=8, space="PSUM") as ps_pool,
        ):
            ident = consts.tile([P, P], _bf16)
            make_identity(nc, ident)
            maskt_sb = consts.tile([P, KT, NB], _bf16)
            nc.sync.dma_start(
                maskt_sb[:], maskt.rearrange("(kt p) b -> p kt b", p=P)
            )

            # resident masked x^T: [128 k-part, KT, MT*128 m-cols]
            xt = xt_pool.tile([P, KT, MT * P], _bf16)

            # ---- phase 1: transpose + mask ----
            loop = tc.For_i(0, repeat, 1) if (repeat > 1 and loop_all) else None
            if loop is not None:
                loop.__enter__()
            for mt in range(MT):
                x_t = xin_pool.tile([P, d_in], _bf16, tag="x_t")
                nc.sync.dma_start(x_t[:], xb[mt * P:(mt + 1) * P, :])
                for kt in range(KT):
                    t_ps = ps_pool.tile([P, n_slice], _f32, tag="ps")
                    tp = t_ps.bitcast(_bf16)
                    nc.tensor.transpose(
                        tp[:, :P], x_t[:, kt * P:(kt + 1) * P], ident[:]
                    )
                    dst = xt[:, kt, mt * P:(mt + 1) * P].rearrange(
                        "p (b c) -> p b c", c=BLOCK)
                    src = tp[:, :P].rearrange("p (b c) -> p b c", c=BLOCK)
                    msk = maskt_sb[:, kt, 2 * mt:2 * mt + 2]
                    nc.vector.tensor_tensor(
                        dst, src, msk[:, :, None].to_broadcast((P, 2, BLOCK)),
                        mybir.AluOpType.mult,
                    )

            # ---- phase 2: sparse col-packed bf16 matmuls, stream W once ----
            if repeat > 1 and not loop_all:
                loop = tc.For_i(0, repeat, 1)
                loop.__enter__()
            for nt in range(NT):
                acc = []
                for mt in range(MT):
                    acc_mt = ps_pool.tile([P, n_slice], _f32, tag="ps",
                                          name=f"acc_{nt}_{mt}")
                    acc.append(acc_mt)
                for kt in range(KT):
                    w_t = w_pool.tile([P, n_slice], _bf16, tag="w_t")
                    nc.sync.dma_start(
                        w_t[:],
                        w[kt * P:(kt + 1) * P, nt * n_slice:(nt + 1) * n_slice],
                    )
                    # emit alternating column groups so adjacent matmuls land
                    # in different PE array halves and dual-issue
                    byg = ([], [])
                    for mt, pair in enumerate(pairs):
                        for g, b in enumerate(pair):
                            if kt in ksets[b]:
                                byg[g].append((mt, g, b))
                    order = []
                    for i in range(max(len(byg[0]), len(byg[1]))):
                        for g in (0, 1):
                            if i < len(byg[g]):
                                order.append(byg[g][i])
                    for mt, g, b in order:
                        lst = klists[b]
                        nc.tensor.matmul(
                            acc[mt][64 * g:64 * g + 64, :],
                            xt[:, kt, b * BLOCK:(b + 1) * BLOCK],
                            w_t[:],
                            start=(kt == lst[0]),
                            stop=(kt == lst[-1]),
                            # paired halves share a bank at disjoint
                            # partitions; sim zero-region is partition-blind
                            skip_group_check=True,
                        )
                for mt, pair in enumerate(pairs):
                    o_t = out_pool.tile([P, n_slice], _f32, tag="o_t")
                    if mt % 4 == 0:
                        nc.vector.tensor_copy(out=o_t[:], in_=acc[mt][:])
                    else:
                        nc.scalar.copy(out=o_t[:], in_=acc[mt][:])
                    for g, b in enumerate(pair):
                        nc.sync.dma_start(
                            y[b * BLOCK:(b + 1) * BLOCK,
                              nt * n_slice:(nt + 1) * n_slice],
                            o_t[64 * g:64 * g + 64, :],
                        )
            if loop is not None:
                loop.__exit__(None, None, None)
    return nc


_cache: dict = {}
MM_DTYPE = "bf16"


def _get_nc(rows, d_in, d_out):
    key = (rows, d_in, d_out, MM_DTYPE)
    if key not in _cache:
        nc = build_kernel(rows, d_in, d_out, mm_dtype=MM_DTYPE)
        # hw-path only: sim bookkeeping predates inserted NoOps
        _split_excess_waits(nc)
        _cache[key] = nc
    return _cache[key]


SPARSE = True


def host_mask(x64: np.ndarray) -> np.ndarray:
    """Exact (f64) block mask for a row shard [rows, d_in]."""
    r, d = x64.shape
    blocks = np.abs(x64.astype(np.float64)).reshape(r // BLOCK, BLOCK, d // BLOCK, BLOCK)
    return blocks.mean(axis=(1, 3)) > 0.8


def _run_percore(ncs, in_maps):
    """Dispatch one program per core asynchronously; return per-core outputs."""
    import jax
    from concourse import bass2jax
    from concourse.bass2jax import _bass_exec_p

    bass2jax.install_neuronx_cc_hook()
    devices = jax.devices()[:len(ncs)]
    outs = []
    for i, (nc, in_map) in enumerate(zip(ncs, in_maps)):
        partition_name = nc.partition_id_tensor.name if nc.partition_id_tensor else None
        in_names, out_names, out_avals, zero_outs = [], [], [], []
        for alloc in nc.m.functions[0].allocations:
            if not isinstance(alloc, mybir.MemoryLocationSet):
                continue
            name = alloc.memorylocations[0].name
            if alloc.kind == "ExternalInput":
                if name != partition_name:
                    in_names.append(name)
            elif alloc.kind == "ExternalOutput":
                shape = tuple(alloc.tensor_shape)
                dtype = mybir.dt.np(alloc.dtype)
                out_names.append(name)
                out_avals.append(jax.core.ShapedArray(shape, dtype))
                zero_outs.append(np.zeros(shape, dtype))
        n_params = len(in_names)
        all_in = in_names + out_names + ([partition_name] if partition_name else [])

        def _body(*args, _nc=nc, _avals=tuple(out_avals), _in=tuple(all_in),
                  _out=tuple(out_names), _pid=partition_name):
            operands = list(args)
            if _pid is not None:
                operands.append(bass2jax.partition_id_tensor())
            return tuple(_bass_exec_p.bind(
                *operands, out_avals=_avals, in_names=_in, out_names=_out,
                lowering_input_output_aliases=(),
                sim_require_finite=True, sim_require_nnan=True, nc=_nc,
            ))

        fn = jax.jit(_body, donate_argnums=tuple(range(n_params, n_params + len(out_names))),
                     keep_unused=True)
        dev = devices[i]
        args = [jax.device_put(np.asarray(in_map[nm]), dev) for nm in in_names]
        args += [jax.device_put(z, dev) for z in zero_outs]
        outs.append((fn(*args), out_names))
    return [{nm: np.asarray(o) for nm, o in zip(names, out)} for out, names in outs]


def kernel(x: np.ndarray, weight: np.ndarray, **run_kwargs):
    import ml_dtypes
    x = np.ascontiguousarray(x, dtype=np.float32)
    weight = np.ascontiguousarray(weight, dtype=np.float32)
    bsz, d_in = x.shape
    d_out = weight.shape[1]
    rows = bsz // N_CORES

    if not SPARSE:
        if MM_DTYPE == "bf16":
            w_in = np.ascontiguousarray(weight.astype(ml_dtypes.bfloat16))
        else:
            w_in = weight
        nc = _get_nc(rows, d_in, d_out)
        in_maps = [
            {"x": x[i * rows:(i + 1) * rows], "w": w_in} for i in range(N_CORES)
        ]
        res = run_bass_kernel_spmd(nc, in_maps, list(range(N_CORES)), **run_kwargs)
        out = np.concatenate([res.results[i]["y"] for i in range(N_CORES)], axis=0)
        if run_kwargs:
            kernel.last_result = res
        return out

    try:
        w_in = np.ascontiguousarray(weight.astype(ml_dtypes.bfloat16))
        ncs, in_maps = [], []
        for i in range(N_CORES):
            xs = x[i * rows:(i + 1) * rows]
            cm = host_mask(xs)                   # [rows//64, d_in//64] bool
            key = ("sparse", rows, d_in, d_out, cm.tobytes())
            if key not in _cache:
                nc = build_sparse(rows, d_in, d_out, cm)
                _split_excess_waits(nc)
                _cache[key] = nc
            ncs.append(_cache[key])
            maskt = np.ascontiguousarray(
                np.repeat(cm.T.astype(ml_dtypes.bfloat16), BLOCK, axis=0))
            in_maps.append({
                "xb": np.ascontiguousarray(xs.astype(ml_dtypes.bfloat16)),
                "w": w_in,
                "maskt": maskt,
            })
        res = _run_percore(ncs, in_maps)
        return np.concatenate([res[i]["y"] for i in range(N_CORES)], axis=0)
    except Exception:
        # fall back to the dense SPMD path
        import traceback
        traceback.print_exc()
        w_in = (np.ascontiguousarray(weight.astype(ml_dtypes.bfloat16))
                if MM_DTYPE == "bf16" else weight)
        nc = _get_nc(rows, d_in, d_out)
        in_maps = [
            {"x": x[i * rows:(i + 1) * rows], "w": w_in} for i in range(N_CORES)
        ]
        res = run_bass_kernel_spmd(nc, in_maps, list(range(N_CORES)))
        return np.concatenate([res.results[i]["y"] for i in range(N_CORES)], axis=0)

